# revision 16
# baseline (speedup 1.0000x reference)
"""Trainium2 Bass kernel for gnn_message_passing (nn_Mesh1_14267881357850).

Reference computation (N=200000, D_SPATIAL=64, D_STRUCT=131, D_OUT=256):
    out1 = concat(spatial, structural) @ W_comb.T + b_comb          [N, 256]
    agg  = (structural + structural[neighbour].sum(1)) * 0.25       [N, 131]
    out2 = agg @ W_agg.T + b_agg                                    [N, 256]
returns (out1, out2)

Strategy (8 cores, node-parallel, bf16 compute / fp32 accumulate):
  * Nodes padded to 200704 and sharded 25088/core = 7 superblocks x
    7 groups x 512 nodes.
  * Neighbour rows are fetched with ONE InstDMAGatherAnt per superblock
    (10752 indices, transpose=True): the gather writes rows FEATURE-MAJOR
    across partitions, so no PE transposes / PSUM staging are needed at
    all.  The int16-index reach limit (32768 rows) is beaten by packing a
    per-superblock gather table host-side: each superblock's 10752 index
    draws hit only ~10.5k distinct rows of `structural`; the host dedups
    them into tables[s] (rows padded to 512 B for the 256 B-multiple elem
    restriction) and remaps indices to table-local int16.  HW quirk
    (measured): the ucode reads the wrapped index array from SBUF
    partitions 16..31, and transpose mode requires single_packet=False.
  * VectorE sums the 3 gathered slabs + the (feature-major, host
    pre-transposed) self rows -> aggA/aggB.
  * Per 512-node group, 8 bf16 matmuls (1 cycle/row vs 4 for fp32) write
    two [128,1024] PSUM tiles; biases ride the matmul via a host ones-row
    in a1T (w1/w2 carry b_comb/b_agg rows); 0.25 is folded into W_agg.
  * ScalarE copies PSUM->SBUF bf16; outputs return bf16 -> fp32 on host
    (rel tolerance 2e-2 >> bf16 rounding).
"""

import os
import sys

import numpy as np

for _p in ("/opt/trn_rl_repo", "/root/.axon_site/_ro/trn_rl_repo"):
    if os.path.isdir(_p) and _p not in sys.path:
        sys.path.append(_p)

import concourse.bacc as bacc
import concourse.bass as bass
import concourse.mybir as mybir
from concourse.bass_utils import run_bass_kernel_spmd
from concourse.tile import TileContext

import ml_dtypes

F32 = mybir.dt.float32
BF16 = mybir.dt.bfloat16
I16 = mybir.dt.int16
NP_BF16 = ml_dtypes.bfloat16

N = 200000
DS = 64           # spatial features
DT = 131          # structural features
DO = 256          # output features per head
NCORES = 8
GROUP = 512       # nodes per pipeline group
SBS = [8, 8, 8, 8, 8, 8, 1]   # groups per superblock (uneven: short tail)
NSB = len(SBS)
NPC = sum(SBS) * GROUP      # 25088 nodes per core
NPAD = NPC * NCORES         # 200704
SBN = [g * GROUP for g in SBS]          # nodes per superblock
SBOFF = [sum(SBN[:i]) for i in range(NSB + 1)]
IOFF = [3 * o // 16 for o in SBOFF]     # idx cols offset per superblock
DMAX = 12544      # gather table rows (max distinct idx per superblock)
ELEM = 256        # gather row elements (bf16) = 512 B

KA = DT + 1 + DS  # 196 rows of a1T ([structT; ones; spatialT])
KB = KA - 128     # 68

# exec time of the last traced run (ns), for test harnesses
last_exec_time_ns = None


def build_nc():
    """Build the Bass module for one core."""
    nc = bacc.Bacc("TRN2", target_bir_lowering=False, debug=False)
    a1T = nc.dram_tensor("a1T", [KA, NPC], BF16, kind="ExternalInput")
    tables = nc.dram_tensor("tables", [NSB, DMAX, ELEM], BF16,
                            kind="ExternalInput")
    idx = nc.dram_tensor("idx", [128, IOFF[NSB]], I16,
                         kind="ExternalInput")
    w1 = nc.dram_tensor("w1", [KA, DO], BF16, kind="ExternalInput")
    w2 = nc.dram_tensor("w2", [DT + 1, DO], BF16, kind="ExternalInput")
    # out[p, k, n]: output feature k*128+p of node n; k in {0,1}=out1,
    # {2,3}=out2
    out = nc.dram_tensor("out", [128, 4, NPC], BF16, kind="ExternalOutput")

    with TileContext(nc) as tc:
        with (
            tc.tile_pool(name="const", bufs=1) as cpool,
            tc.tile_pool(name="gather", bufs=2) as gpool,
            tc.tile_pool(name="acts", bufs=2) as apool,
            tc.tile_pool(name="work", bufs=3) as wpool,
            tc.tile_pool(name="osb", bufs=3) as opool,
            tc.tile_pool(name="pout", bufs=4, space="PSUM") as pout,
        ):
            # ---- constants ----
            w1a = cpool.tile([128, DO], BF16)
            nc.sync.dma_start(out=w1a, in_=w1[0:128, :])
            w1b = cpool.tile([KB, DO], BF16)
            nc.sync.dma_start(out=w1b, in_=w1[128:KA, :])
            w2a = cpool.tile([128, DO], BF16)
            nc.sync.dma_start(out=w2a, in_=w2[0:128, :])
            w2b = cpool.tile([4, DO], BF16)
            nc.sync.dma_start(out=w2b, in_=w2[128:DT + 1, :])
            # idx is loaded per-superblock so gather 0 waits only on its own
            # 43KB slice rather than the whole 301KB array (trims pipeline
            # startup; Pool is the critical path from the first gather on)
            idx_sb = cpool.tile([128, IOFF[NSB]], I16)
            for s_ in range(NSB):
                nc.sync.dma_start(
                    out=idx_sb[:, IOFF[s_]:IOFF[s_ + 1]],
                    in_=idx[:, IOFF[s_]:IOFF[s_ + 1]])

            def issue_loads(s):
                n = SBN[s]
                nsl = slice(SBOFF[s], SBOFF[s] + n)
                jcols = n // 16
                # one gather per neighbour slot (num_idxs capped ~7680 by the
                # Q7 ucode's index scratch; larger merged gathers measured
                # slower via descriptor-ring backpressure)
                gts = []
                for j in range(3):
                    gt = gpool.tile([128, 2, n], BF16, tag=f"gt{j}_{n}")
                    c0 = IOFF[s] + j * jcols
                    nc.gpsimd.dma_gather(
                        out_ap=gt[:, :, :],
                        in_ap=tables[s, :, :],
                        idxs_ap=idx_sb[:, c0:c0 + jcols],
                        num_idxs=n,
                        num_idxs_reg=n,
                        elem_size=ELEM,
                        transpose=True,
                        single_packet=False,
                    )
                    gts.append(gt)
                a1a7 = apool.tile([128, n], BF16, tag=f"a1a_{n}")
                nc.sync.dma_start(out=a1a7, in_=a1T[0:128, nsl])
                a1b7 = apool.tile([KB, n], BF16, tag=f"a1b_{n}")
                nc.sync.dma_start(out=a1b7, in_=a1T[128:KA, nsl])
                return gts, a1a7, a1b7

            def compute_sb(s, gts, a1a7, a1b7):
                for gl in range(SBS[s]):
                    n0 = SBOFF[s] + gl * GROUP
                    gsl = slice(gl * GROUP, (gl + 1) * GROUP)
                    # 3-neighbour sum, feature-major (both 128-chunks at once)
                    nsum = wpool.tile([128, 2, GROUP], BF16, tag="nsum")
                    o0 = gl * GROUP
                    nc.vector.tensor_add(
                        out=nsum,
                        in0=gts[0][:, :, o0:o0 + GROUP],
                        in1=gts[1][:, :, o0:o0 + GROUP])
                    nc.vector.tensor_add(
                        out=nsum, in0=nsum,
                        in1=gts[2][:, :, o0:o0 + GROUP])
                    # + self rows (already feature-major in a1T)
                    aggA = wpool.tile([128, GROUP], BF16, tag="aggA")
                    nc.vector.tensor_add(
                        out=aggA, in0=nsum[:, 0, :], in1=a1a7[:, gsl])
                    # rows 0..2: struct feats 128..130 (+0 pad row 3 in nsum);
                    # row 3 of a1b is the host ones-row -> bias via w matmul
                    aggB = wpool.tile([4, GROUP], BF16, tag="aggB")
                    nc.vector.tensor_add(
                        out=aggB, in0=nsum[0:4, 1, :], in1=a1b7[0:4, gsl])

                    p1 = pout.tile([128, 2 * GROUP], F32, tag="ps")
                    p2 = pout.tile([128, 2 * GROUP], F32, tag="ps")
                    for c in range(2):
                        cs = slice(c * 128, (c + 1) * 128)
                        os_ = slice(c * GROUP, (c + 1) * GROUP)
                        nc.tensor.matmul(
                            p1[:, os_], lhsT=w1a[:, cs], rhs=a1a7[:, gsl],
                            start=True, stop=False)
                        nc.tensor.matmul(
                            p1[:, os_], lhsT=w1b[:, cs], rhs=a1b7[:, gsl],
                            start=False, stop=True)
                        nc.tensor.matmul(
                            p2[:, os_], lhsT=w2a[:, cs], rhs=aggA,
                            start=True, stop=False)
                        nc.tensor.matmul(
                            p2[:, os_], lhsT=w2b[:, cs], rhs=aggB,
                            start=False, stop=True)
                    osb = opool.tile([128, 4 * GROUP], BF16, tag="osb")
                    nc.scalar.copy(out=osb[:, 0:2 * GROUP], in_=p1)
                    nc.scalar.copy(out=osb[:, 2 * GROUP:4 * GROUP], in_=p2)
                    nc.sync.dma_start(
                        out=out[:, :, n0:n0 + GROUP],
                        in_=osb.rearrange("p (k n) -> p k n", k=4))

            pend = None
            for s in range(NSB):
                tiles = issue_loads(s)
                if pend is not None:
                    compute_sb(pend[0], *pend[1])
                pend = (s, tiles)
            compute_sb(pend[0], *pend[1])
    nc.compile()
    return nc


def prep_inputs(spatial, structural, neighbour, W_agg, b_agg, W_comb, b_comb):
    """Host-side shard + layout transform. Returns list of per-core in_maps."""
    n = spatial.shape[0]

    spatial = np.asarray(spatial, dtype=np.float32)
    structural = np.asarray(structural, dtype=np.float32)
    nbr = np.asarray(neighbour, dtype=np.int64)

    pad = NPAD - n
    if pad:
        spatial_p = np.concatenate(
            [spatial, np.zeros((pad, DS), np.float32)], axis=0)
        structural_p = np.concatenate(
            [structural, np.zeros((pad, DT), np.float32)], axis=0)
        nbr_p = np.concatenate([nbr, np.zeros((pad, 3), np.int64)], axis=0)
    else:
        spatial_p, structural_p, nbr_p = spatial, structural, nbr

    s_bf16 = structural.astype(NP_BF16)

    # a1T rows: [structT(0..130); ones(131); spatialT(132..195)]
    a1T_all = np.empty((KA, NPAD), NP_BF16)
    a1T_all[0:DT] = structural_p.T.astype(NP_BF16)
    a1T_all[DT] = NP_BF16(1.0)
    a1T_all[DT + 1:KA] = spatial_p.T.astype(NP_BF16)

    Wc = np.asarray(W_comb, np.float32)   # [256, 195]; cols 0..63 spatial
    w1 = np.empty((KA, DO), NP_BF16)
    w1[0:DT] = Wc[:, DS:DS + DT].T.astype(NP_BF16)
    w1[DT] = np.asarray(b_comb, np.float32).astype(NP_BF16)
    w1[DT + 1:KA] = Wc[:, 0:DS].T.astype(NP_BF16)
    Wa = np.asarray(W_agg, np.float32)    # [256, 131]
    w2 = np.empty((DT + 1, DO), NP_BF16)
    w2[0:DT] = (0.25 * Wa.T).astype(NP_BF16)
    w2[DT] = np.asarray(b_agg, np.float32).astype(NP_BF16)

    in_maps = []
    for c in range(NCORES):
        tables = np.zeros((NSB, DMAX, ELEM), NP_BF16)
        idx_w = np.zeros((128, IOFF[NSB]), np.int16)
        for s in range(NSB):
            nsb = SBN[s]
            n0 = c * NPC + SBOFF[s]
            nbr_sb = nbr_p[n0:n0 + nsb]             # [nsb, 3]
            uniq, inv = np.unique(nbr_sb, return_inverse=True)
            assert uniq.size <= DMAX, f"DMAX too small: {uniq.size}"
            tables[s, :uniq.size, 0:DT] = s_bf16[uniq]
            inv = inv.reshape(nsb, 3)
            jcols = nsb // 16
            for j in range(3):
                # gather j's slot i = node i; wrapped at partitions 16..31
                flat = np.ascontiguousarray(inv[:, j]).astype(np.int16)
                c0 = IOFF[s] + j * jcols
                idx_w[16:32, c0:c0 + jcols] = flat.reshape(jcols, 16).T
        in_maps.append({
            "a1T": np.ascontiguousarray(a1T_all[:, c * NPC:(c + 1) * NPC]),
            "tables": tables,
            "idx": idx_w,
            "w1": w1,
            "w2": w2,
        })
    return in_maps


_NC_CACHE = {}


def kernel(spatial, structural, neighbour, W_agg, b_agg, W_comb, b_comb):
    global last_exec_time_ns
    if "nc" not in _NC_CACHE:
        _NC_CACHE["nc"] = build_nc()
    nc = _NC_CACHE["nc"]

    in_maps = prep_inputs(
        spatial, structural, neighbour, W_agg, b_agg, W_comb, b_comb)

    trace = bool(int(os.environ.get("KERNEL_TRACE", "0")))
    tmpdir = os.environ.get("KERNEL_TMPDIR") or None
    res = run_bass_kernel_spmd(
        nc, in_maps, core_ids=list(range(NCORES)), trace=trace, tmpdir=tmpdir)
    last_exec_time_ns = res.exec_time_ns

    # res out: [128, 4, npc] bf16 per core
    parts = [np.asarray(r["out"]) for r in res.results]
    comb = np.concatenate(parts, axis=2).astype(np.float32)  # [128, 4, npad]
    out1 = np.concatenate([comb[:, 0, :], comb[:, 1, :]], axis=0)[:, :N].T
    out2 = np.concatenate([comb[:, 2, :], comb[:, 3, :]], axis=0)[:, :N].T
    return np.ascontiguousarray(out1), np.ascontiguousarray(out2)
